# revision 36
# baseline (speedup 1.0000x reference)
"""Distributed multi-head attention kernel for 8 TRN2 NeuronCores.

Sharding: tensor-parallel over heads (2 heads/core) for the qkv projection
and attention; two half-sized AllToAll exchanges (one per batch item, the
first hidden under batch-1 compute) carrying the unnormalized attention
output plus softmax denominators; row-parallel output projection; host
reassembles (core j owns 256-query half-block j of each batch item).

Numerics tricks vs the straightforward version:
- k-bias dropped entirely (adds a per-query constant to every logit of that
  query -> softmax invariant).
- v-bias folded into the output projection bias on the host
  (o = softmax(s) @ (v + bv) -> out = o' @ wo + (bv @ wo + bo)).
- softmax division deferred through the AllToAll: each core ships
  sum_k exp(s)·v (unnormalized) plus the denominators (2 extra rows per
  130-row slice); the receiving side multiplies by 1/denom (broadcast via a
  selector matmul) before the output projection.
"""

import sys

sys.path.insert(0, "/opt/trn_rl_repo")

import ml_dtypes
import numpy as np

# Problem constants (hardcoded per harness contract)
B = 2
S = 2048
DIM = 1024
N_HEAD = 16
HD = 64  # head dim
SCALE = HD ** (-0.5)
R = B * S  # 4096 flattened rows
NCORES = 8
HPC = N_HEAD // NCORES  # 2 heads per core
FPC = HPC * HD  # 128 features per core
RPC = R // NCORES  # 512 rows per core (output row slice)
HB = 256  # queries per half-block (a2a slice unit)

KT = DIM // 128  # 8 k-tiles over the model dim
NKT = S // 128  # 16 key tiles per sequence
NQB = S // 512  # 4 query blocks per sequence
SLICE = FPC + HPC  # 130 rows per a2a slice: 128 features + 2 denom rows

_CACHED = {}


def _build_graph():
    import concourse.mybir as mybir
    import concourse.tile as tile
    from concourse import bacc
    from concourse.masks import make_identity

    # This kernel only uses Exp and Ln activations. Both live in the
    # "natural_log_exp_and_others" ACT table set, but the table-load pass
    # maps each function to the first set containing it, which puts Exp in
    # "exp_and_others" and forces a ~1.3us table reload around every Ln.
    # Restrict the table list for this build so both resolve to the same
    # set and the table loads exactly once.
    real_get_tables = bacc.get_activation_tables

    def _tables_ln_exp_merged(arch):
        tables = dict(real_get_tables(arch))
        if "natural_log_exp_and_others" in tables and "exp_and_others" in tables:
            tables["exp_and_others"] = set()
        return tables

    nc = bacc.Bacc(
        "TRN2",
        target_bir_lowering=False,
        debug=False,
        num_devices=NCORES,
    )

    bacc.get_activation_tables = _tables_ln_exp_merged
    try:
        _nc = _build_body(nc, mybir, tile, make_identity)
    finally:
        bacc.get_activation_tables = real_get_tables
    return _nc


def _build_body(nc, mybir, tile, make_identity):
    f32 = mybir.dt.float32
    f32r = mybir.dt.float32r
    bf16 = mybir.dt.bfloat16
    EXP = mybir.ActivationFunctionType.Exp
    LN = mybir.ActivationFunctionType.Ln

    xt = nc.dram_tensor("xt", [DIM, R], bf16, kind="ExternalInput").ap()
    wqkv = nc.dram_tensor("wqkv", [DIM, 3 * FPC], bf16, kind="ExternalInput").ap()
    bq = nc.dram_tensor("bq", [1, FPC], f32, kind="ExternalInput").ap()
    wo = nc.dram_tensor("wo", [DIM, DIM], bf16, kind="ExternalInput").ap()
    bo = nc.dram_tensor("bo", [8, 128], f32, kind="ExternalInput").ap()
    selm = nc.dram_tensor("selm", [16, KT * 128], f32, kind="ExternalInput").ap()
    out = nc.dram_tensor("out", [DIM, RPC], f32, kind="ExternalOutput").ap()

    with tile.TileContext(nc) as tc:
        with (
            tc.tile_pool(name="glob", bufs=1) as glob,
            tc.tile_pool(name="xTp", bufs=2) as xT_pool,
            tc.tile_pool(name="vtb", bufs=2) as vt_pool,
            tc.tile_pool(name="ptp", bufs=3) as pt_pool,
            tc.tile_pool(name="obp", bufs=2) as ob_pool,
            tc.tile_pool(name="outp", bufs=2) as out_pool,
            tc.tile_pool(name="dram", bufs=1, space="DRAM") as dram_pool,
            tc.tile_pool(name="pspp", bufs=2, space="PSUM") as ps_pp,
            tc.tile_pool(name="psst", bufs=2, space="PSUM") as ps_st,
            tc.tile_pool(name="pso", bufs=1, space="PSUM") as ps_o,
        ):
            # ---------- stage 0: weights / biases / constants ----------
            wqkv_sb = []
            for k in range(KT):
                w_t = glob.tile([128, 3 * FPC], bf16, name=f"w_{k}")
                nc.gpsimd.dma_start(out=w_t[:], in_=wqkv[k * 128 : (k + 1) * 128, :])
                wqkv_sb.append(w_t)
            bias_q = glob.tile([128, 1], f32)
            nc.gpsimd.dma_start(out=bias_q[:, 0:1], in_=bq[0:1, :])
            bias_o = glob.tile([128, 8], f32)
            for m in range(8):
                nc.gpsimd.dma_start(out=bias_o[:, m : m + 1], in_=bo[m : m + 1, :])

            ident32 = glob.tile([128, 128], f32)
            make_identity(nc, ident32)

            # persistent activations (bf16 compute operands)
            qT = glob.tile([128, R], bf16)
            kT = glob.tile([128, R], bf16)
            # v natural layout + ones column: per (b, hh, kt) a [128, 65] block
            v_nat = glob.tile([128, B * HPC * NKT * 65], bf16)
            ones_tmp = glob.tile([128, 64], f32)
            nc.vector.memset(ones_tmp[:], 1.0)
            nc.vector.tensor_copy(out=v_nat[:, 64::65], in_=ones_tmp[:])
            # selector matrices for the denominator broadcast (host-built):
            # sel[k].T @ recip replicates recip row 2k over partitions 0..63
            # and row 2k+1 over 64..127
            sel_f = glob.tile([16, KT * 128], f32, name="sel_f")
            nc.gpsimd.dma_start(out=sel_f[:], in_=selm[:, :])
            sel_all = glob.tile([16, KT * 128], f32r, name="sel_all")
            nc.vector.tensor_copy(out=sel_all[:], in_=sel_f[:])
            sel = [sel_all[:, k * 128 : (k + 1) * 128] for k in range(KT)]

            # collective buffers + warmup (fires the ncfw path early)
            warm_in = dram_pool.tile([NCORES, 16], bf16, name="warm_in")
            warm_out = dram_pool.tile([NCORES, 16], bf16, name="warm_out")
            nc.gpsimd.dma_start(out=warm_in[0:1, :], in_=ones_tmp[0:1, 0:16])
            nc.gpsimd.dma_start(
                out=warm_in[1:NCORES, :],
                in_=warm_in[0:1, :].to_broadcast((NCORES - 1, 16)),
            )
            nc.gpsimd.collective_compute(
                "AllToAll",
                mybir.AluOpType.bypass,
                replica_groups=[list(range(NCORES))],
                ins=[warm_in[:].opt()],
                outs=[warm_out[:].opt()],
            )
            a2a_in = [
                dram_pool.tile([NCORES * SLICE, HB], bf16, name=f"a2a_in{b}")
                for b in range(B)
            ]
            a2a_out = [
                dram_pool.tile([NCORES * SLICE, HB], bf16, name=f"a2a_out{b}")
                for b in range(B)
            ]

            # ---------- helpers ----------
            def emit_xt_dmas(g, split=False, halves=False):
                xT_g = [
                    xT_pool.tile([128, 1024], bf16, name=f"xT_{k}", tag=f"xT{k}")
                    for k in range(KT)
                ]
                if halves:
                    # two half-column DMAs per chunk: the h=0 projection
                    # chains only wait for the first half-sized transfers
                    for h in range(2):
                        for k in range(KT):
                            eng = nc.scalar if k % 2 else nc.sync
                            c0 = g * 1024 + h * 512
                            eng.dma_start(
                                out=xT_g[k][:, h * 512 : (h + 1) * 512],
                                in_=xt[k * 128 : (k + 1) * 128, c0 : c0 + 512],
                            )
                    return xT_g
                for k in range(KT):
                    # split=True alternates trigger queues (SP/ACT HWDGE) to
                    # halve the ~600ns-per-DMA descriptor-gen serialization
                    eng = nc.scalar if (split and k % 2) else nc.sync
                    eng.dma_start(
                        out=xT_g[k][:],
                        in_=xt[k * 128 : (k + 1) * 128, g * 1024 : (g + 1) * 1024],
                    )
                return xT_g

            def emit_chain(g, h, m, xT_g):
                # one projection chain (512 rows, one of q/k/v) + v transposes
                col0 = g * 1024 + h * 512
                pp = ps_pp.tile([128, 512], f32, name="pp", tag="pp")
                for kc in range(KT):
                    nc.tensor.matmul(
                        pp[:],
                        lhsT=wqkv_sb[kc][:, m * 128 : (m + 1) * 128],
                        rhs=xT_g[kc][:, h * 512 : (h + 1) * 512],
                        start=(kc == 0),
                        stop=(kc == KT - 1),
                    )
                if m == 1:
                    nc.vector.tensor_copy(out=kT[:, col0 : col0 + 512], in_=pp[:])
                elif m == 0:
                    nc.vector.tensor_scalar_add(
                        out=qT[:, col0 : col0 + 512],
                        in0=pp[:],
                        scalar1=bias_q[:, 0:1],
                    )
                else:
                    vT_blk = vt_pool.tile(
                        [128, 512], f32, name="vT_blk", tag="vT_blk"
                    )
                    nc.vector.tensor_copy(out=vT_blk[:], in_=pp[:])
                    for j in range(4):
                        row0 = col0 + j * 128
                        b_idx = row0 // S
                        kt = (row0 % S) // 128
                        pstv = ps_pp.tile([128, 512], f32, name="pstv", tag="pp")
                        nc.tensor.transpose(
                            pstv[:, 0:128],
                            vT_blk[:, j * 128 : (j + 1) * 128],
                            ident32[:],
                        )
                        for hh in range(HPC):
                            col = ((b_idx * HPC + hh) * NKT + kt) * 65
                            nc.vector.tensor_copy(
                                out=v_nat[:, col : col + 64],
                                in_=pstv[:, hh * 64 : (hh + 1) * 64],
                            )

            def emit_attn_block(b, qb, mid=None):
                # mid: {kt: callable} emission callbacks interleaved into the
                # kt loop (just-in-time projection chains for later blocks)
                q0 = b * S + qb * 512
                po = ps_o.tile([65, 1024], f32, name="po", tag="po")
                for kt in range(NKT):
                    if mid and kt in mid:
                        mid[kt]()
                    k0 = b * S + kt * 128
                    pst = ps_st.tile([128, 1024], f32, name="pst", tag="pst")
                    for hh in range(HPC):
                        nc.tensor.matmul(
                            pst[:, hh * 512 : (hh + 1) * 512],
                            lhsT=kT[hh * 64 : (hh + 1) * 64, k0 : k0 + 128],
                            rhs=qT[hh * 64 : (hh + 1) * 64, q0 : q0 + 512],
                            start=True,
                            stop=True,
                            tile_position=(hh * 64, 0),
                        )
                    ptile = pt_pool.tile([128, 1024], bf16, name="ptile", tag="ptile")
                    nc.scalar.activation(ptile[:], pst[:], EXP, scale=SCALE)
                    for hh in range(HPC):
                        col = ((b * HPC + hh) * NKT + kt) * 65
                        nc.tensor.matmul(
                            po[:, hh * 512 : (hh + 1) * 512],
                            lhsT=v_nat[:, col : col + 65],
                            rhs=ptile[:, hh * 512 : (hh + 1) * 512],
                            start=(kt == 0),
                            stop=(kt == NKT - 1),
                        )
                # evacuate unnormalized block + denominators into the a2a
                # input, split into two 256-query half-block slices
                obk = ob_pool.tile([65, 1024], bf16, name="obk", tag="obk")
                nc.vector.tensor_copy(out=obk[:], in_=po[:])
                buf = a2a_in[b]
                # all staging on the SP queue: the POOL queue must stay clear
                # of work that queues behind a collective's completion-wait,
                # and the ACT queue must stay clear during attention
                for s in range(2):
                    r0 = (qb * 2 + s) * SLICE
                    c0 = s * HB
                    nc.sync.dma_start(
                        out=buf[r0 : r0 + 64, :], in_=obk[0:64, c0 : c0 + HB]
                    )
                    nc.sync.dma_start(
                        out=buf[r0 + 64 : r0 + 128, :],
                        in_=obk[0:64, 512 + c0 : 512 + c0 + HB],
                    )
                    nc.sync.dma_start(
                        out=buf[r0 + 128 : r0 + 129, :], in_=obk[64:65, c0 : c0 + HB]
                    )
                    nc.sync.dma_start(
                        out=buf[r0 + 129 : r0 + 130, :],
                        in_=obk[64:65, 512 + c0 : 512 + c0 + HB],
                    )
                return obk

            def emit_outproj(ph):
                # one 256-query half: loads, reciprocal, broadcast, scale,
                # projection. Trigger queues are spread (SP/ACT/POOL) because
                # each [128,x] dma_start costs ~600ns of descriptor-gen.
                src = a2a_out[ph]
                c0 = ph * HB
                # ph 0 runs during attention: keep its triggers off the ACT
                # queue. ph 1 runs in the idle tail: spread across SP + ACT.
                tail = ph == 1
                oTs = []
                for k in range(KT):
                    o_t = glob.tile([128, HB], bf16, name=f"oTs_{ph}_{k}")
                    eng = nc.scalar if (tail and k % 2) else nc.sync
                    eng.dma_start(
                        out=o_t[:], in_=src[k * SLICE : k * SLICE + 128, :]
                    )
                    oTs.append(o_t)
                den = glob.tile([16, HB], bf16, name=f"den_{ph}")
                for k in range(KT):
                    eng = nc.scalar if (tail and k % 2 == 0) else nc.sync
                    eng.dma_start(
                        out=den[k * HPC : (k + 1) * HPC, :],
                        in_=src[k * SLICE + 128 : k * SLICE + 130, :],
                    )
                lden = glob.tile([16, HB], f32, name=f"lden_{ph}")
                nc.scalar.activation(lden[:], den[:], LN)
                recip = glob.tile([16, HB], f32r, name=f"recip_{ph}")
                with nc.allow_low_precision(reason="softmax denom reciprocal"):
                    nc.scalar.activation(recip[:], lden[:], EXP, scale=-1.0)
                for k in range(KT):
                    pbc = ps_pp.tile([128, 512], f32, name="pbc", tag="pp")
                    nc.tensor.matmul(
                        pbc[:, 0:HB],
                        lhsT=sel[k],
                        rhs=recip[:],
                        start=True,
                        stop=True,
                    )
                    nc.vector.tensor_mul(
                        out=oTs[k][:], in0=oTs[k][:], in1=pbc[:, 0:HB]
                    )
                for m in range(8):
                    pout = ps_pp.tile([128, 512], f32, name="pout", tag="pp")
                    for k in range(KT):
                        nc.tensor.matmul(
                            pout[:, 0:HB],
                            lhsT=wo_sb[k][:, m * 128 : (m + 1) * 128],
                            rhs=oTs[k][:],
                            start=(k == 0),
                            stop=(k == KT - 1),
                        )
                    o_sb = out_pool.tile([128, 512], f32, name="o_sb", tag="o_sb")
                    nc.vector.tensor_scalar_add(
                        out=o_sb[:, 0:HB], in0=pout[:, 0:HB],
                        scalar1=bias_o[:, m : m + 1],
                    )
                    eng = nc.scalar if (tail and m % 2) else nc.sync
                    eng.dma_start(
                        out=out[m * 128 : (m + 1) * 128, c0 : c0 + HB],
                        in_=o_sb[:, 0:HB],
                    )

            # ---------- stage 1: proj b0 + attention b0 --------------------
            xg0 = emit_xt_dmas(0, halves=True)
            xg1 = emit_xt_dmas(1, split=True)
            for g in (0, 1):
                for h in range(2):
                    for m in (1, 2, 0):
                        emit_chain(g, h, m, xg0 if g == 0 else xg1)
            xg2 = emit_xt_dmas(2)
            xg3 = emit_xt_dmas(3)
            for qb in range(NQB):
                emit_attn_block(0, qb)
            # wo loads trigger during attention (POOL queue is idle here)
            wo_sb = []
            for k in range(KT):
                w_t = glob.tile([128, DIM], bf16, name=f"wo_{k}")
                nc.gpsimd.dma_start(out=w_t[:], in_=wo[k * 128 : (k + 1) * 128, :])
                wo_sb.append(w_t)
            nc.gpsimd.collective_compute(
                "AllToAll",
                mybir.AluOpType.bypass,
                replica_groups=[list(range(NCORES))],
                ins=[a2a_in[0][:].opt()],
                outs=[a2a_out[0][:].opt()],
            )

            # ---------- stage 2: proj b1 + attention b1, outproj#0 hidden --
            for g in (2, 3):
                for h in range(2):
                    for m in (1, 2, 0):
                        emit_chain(g, h, m, xg2 if g == 2 else xg3)
            emit_attn_block(1, 0)
            emit_attn_block(1, 1)
            emit_attn_block(1, 2)
            emit_outproj(0)
            emit_attn_block(1, 3)

            # ---------- stage 4: final exchange + outproj#1 ----------------
            nc.gpsimd.collective_compute(
                "AllToAll",
                mybir.AluOpType.bypass,
                replica_groups=[list(range(NCORES))],
                ins=[a2a_in[1][:].opt()],
                outs=[a2a_out[1][:].opt()],
            )
            emit_outproj(1)

    nc.compile()
    return nc


def _get_graph():
    if "nc" not in _CACHED:
        _CACHED["nc"] = _build_graph()
    return _CACHED["nc"]


def _make_in_maps(x, wqkv, bqkv, wo, bo):
    bf = ml_dtypes.bfloat16
    x2 = np.asarray(x, dtype=np.float32).reshape(R, DIM)
    xt = np.ascontiguousarray(x2.T.astype(bf))  # [dim, b*s] bf16
    wqkv = np.asarray(wqkv, dtype=np.float32)
    bqkv = np.asarray(bqkv, dtype=np.float32)
    wo_f = np.asarray(wo, dtype=np.float32)
    wo16 = np.ascontiguousarray(wo_f.astype(bf))
    # fold the v-bias through the output projection (k-bias is dropped:
    # softmax-invariant)
    bv = bqkv[2 * DIM : 3 * DIM]
    bo_eff = np.asarray(bo, dtype=np.float32) + bv @ wo_f
    bo_f = np.ascontiguousarray(bo_eff.reshape(8, 128))
    # denominator-broadcast selectors: sel[k].T maps recip rows (2k, 2k+1)
    # onto output partitions (0..63, 64..127)
    selm = np.zeros((16, KT * 128), dtype=np.float32)
    for k in range(KT):
        selm[2 * k, k * 128 : k * 128 + 64] = 1.0
        selm[2 * k + 1, k * 128 + 64 : (k + 1) * 128] = 1.0
    selm = np.ascontiguousarray(selm)

    in_maps = []
    for c in range(NCORES):
        w_s = np.ascontiguousarray(
            np.concatenate(
                [
                    wqkv[:, c * FPC : (c + 1) * FPC],
                    wqkv[:, DIM + c * FPC : DIM + (c + 1) * FPC],
                    wqkv[:, 2 * DIM + c * FPC : 2 * DIM + (c + 1) * FPC],
                ],
                axis=1,
            ).astype(bf)
        )
        b_s = np.ascontiguousarray(
            bqkv[c * FPC : (c + 1) * FPC].reshape(1, FPC)
        )
        in_maps.append(
            {"xt": xt, "wqkv": w_s, "bq": b_s, "wo": wo16, "bo": bo_f, "selm": selm}
        )
    return in_maps


def _assemble_outs(outs):
    # core j's columns: [0:256] = b0 rows j*256..(j+1)*256,
    #                   [256:512] = b1 rows j*256..(j+1)*256
    full = np.empty((R, DIM), dtype=np.float32)
    for j in range(NCORES):
        full[j * HB : (j + 1) * HB, :] = outs[j][:, 0:HB].T
        full[S + j * HB : S + (j + 1) * HB, :] = outs[j][:, HB : 2 * HB].T
    return np.ascontiguousarray(full.reshape(B, S, DIM))


def kernel(x, wqkv, bqkv, wo, bo):
    from concourse.bass_utils import run_bass_kernel_spmd

    nc = _get_graph()
    in_maps = _make_in_maps(x, wqkv, bqkv, wo, bo)
    res = run_bass_kernel_spmd(nc, in_maps, core_ids=list(range(NCORES)))
    outs = [res.results[c]["out"] for c in range(NCORES)]  # each [1024, 512]
    return _assemble_outs(outs)


# revision 39
# speedup vs baseline: 1.1025x; 1.1025x over previous
"""Distributed multi-head attention kernel for 8 TRN2 NeuronCores.

Sharding: tensor-parallel over heads (2 heads/core) for the qkv projection
and attention; two half-sized AllToAll exchanges (one per batch item, the
first hidden under batch-1 compute) carrying the unnormalized attention
output plus softmax denominators; row-parallel output projection; host
reassembles (core j owns 256-query half-block j of each batch item).

Numerics tricks vs the straightforward version:
- k-bias dropped entirely (adds a per-query constant to every logit of that
  query -> softmax invariant).
- v-bias folded into the output projection bias on the host
  (o = softmax(s) @ (v + bv) -> out = o' @ wo + (bv @ wo + bo)).
- softmax division deferred through the AllToAll: each core ships
  sum_k exp(s)·v (unnormalized) plus the denominators (2 extra rows per
  130-row slice); the receiving side multiplies by 1/denom (broadcast via a
  selector matmul) before the output projection.
"""

import sys

sys.path.insert(0, "/opt/trn_rl_repo")

import ml_dtypes
import numpy as np

# Problem constants (hardcoded per harness contract)
B = 2
S = 2048
DIM = 1024
N_HEAD = 16
HD = 64  # head dim
SCALE = HD ** (-0.5)
R = B * S  # 4096 flattened rows
NCORES = 8
HPC = N_HEAD // NCORES  # 2 heads per core
FPC = HPC * HD  # 128 features per core
RPC = R // NCORES  # 512 rows per core (output row slice)
HB = 256  # queries per half-block (a2a slice unit)

KT = DIM // 128  # 8 k-tiles over the model dim
NKT = S // 128  # 16 key tiles per sequence
NQB = S // 512  # 4 query blocks per sequence
SLICE = FPC + HPC  # 130 rows per a2a slice: 128 features + 2 denom rows

_CACHED = {}


def _build_graph():
    import concourse.mybir as mybir
    import concourse.tile as tile
    from concourse import bacc
    from concourse.masks import make_identity

    # This kernel only uses Exp and Ln activations. Both live in the
    # "natural_log_exp_and_others" ACT table set, but the table-load pass
    # maps each function to the first set containing it, which puts Exp in
    # "exp_and_others" and forces a ~1.3us table reload around every Ln.
    # Restrict the table list for this build so both resolve to the same
    # set and the table loads exactly once.
    real_get_tables = bacc.get_activation_tables

    def _tables_ln_exp_merged(arch):
        tables = dict(real_get_tables(arch))
        if "natural_log_exp_and_others" in tables and "exp_and_others" in tables:
            tables["exp_and_others"] = set()
        return tables

    nc = bacc.Bacc(
        "TRN2",
        target_bir_lowering=False,
        debug=False,
        num_devices=NCORES,
    )

    bacc.get_activation_tables = _tables_ln_exp_merged
    try:
        _nc = _build_body(nc, mybir, tile, make_identity)
    finally:
        bacc.get_activation_tables = real_get_tables
    return _nc


def _build_body(nc, mybir, tile, make_identity):
    f32 = mybir.dt.float32
    f32r = mybir.dt.float32r
    bf16 = mybir.dt.bfloat16
    EXP = mybir.ActivationFunctionType.Exp
    LN = mybir.ActivationFunctionType.Ln

    xt = nc.dram_tensor("xt", [DIM, R], bf16, kind="ExternalInput").ap()
    wqkv = nc.dram_tensor("wqkv", [DIM, 3 * FPC], bf16, kind="ExternalInput").ap()
    bq = nc.dram_tensor("bq", [1, FPC], f32, kind="ExternalInput").ap()
    wo = nc.dram_tensor("wo", [DIM, DIM], bf16, kind="ExternalInput").ap()
    bo = nc.dram_tensor("bo", [8, 128], f32, kind="ExternalInput").ap()
    selm = nc.dram_tensor("selm", [16, KT * 128], f32, kind="ExternalInput").ap()
    out = nc.dram_tensor("out", [DIM, RPC], f32, kind="ExternalOutput").ap()

    with tile.TileContext(nc) as tc:
        with (
            tc.tile_pool(name="glob", bufs=1) as glob,
            tc.tile_pool(name="xTp", bufs=2) as xT_pool,
            tc.tile_pool(name="vtb", bufs=2) as vt_pool,
            tc.tile_pool(name="ptp", bufs=3) as pt_pool,
            tc.tile_pool(name="obp", bufs=2) as ob_pool,
            tc.tile_pool(name="outp", bufs=2) as out_pool,
            tc.tile_pool(name="dram", bufs=1, space="DRAM") as dram_pool,
            tc.tile_pool(name="pspp", bufs=2, space="PSUM") as ps_pp,
            tc.tile_pool(name="psst", bufs=2, space="PSUM") as ps_st,
            tc.tile_pool(name="pso", bufs=1, space="PSUM") as ps_o,
        ):
            # ---------- stage 0: weights / biases / constants ----------
            wqkv_sb = []
            for k in range(KT):
                w_t = glob.tile([128, 3 * FPC], bf16, name=f"w_{k}")
                nc.gpsimd.dma_start(out=w_t[:], in_=wqkv[k * 128 : (k + 1) * 128, :])
                wqkv_sb.append(w_t)
            bias_q = glob.tile([128, 1], f32)
            nc.gpsimd.dma_start(out=bias_q[:, 0:1], in_=bq[0:1, :])
            bias_o = glob.tile([128, 8], f32)
            for m in range(8):
                nc.gpsimd.dma_start(out=bias_o[:, m : m + 1], in_=bo[m : m + 1, :])

            ident32 = glob.tile([128, 128], f32)
            make_identity(nc, ident32)

            # persistent activations (bf16 compute operands)
            qT = glob.tile([128, R], bf16)
            kT = glob.tile([128, R], bf16)
            # v natural layout + ones column: per (b, hh, kt) a [128, 65] block
            v_nat = glob.tile([128, B * HPC * NKT * 65], bf16)
            ones_tmp = glob.tile([128, 64], f32)
            nc.vector.memset(ones_tmp[:], 1.0)
            nc.vector.tensor_copy(out=v_nat[:, 64::65], in_=ones_tmp[:])
            # selector matrices for the denominator broadcast (host-built):
            # sel[k].T @ recip replicates recip row 2k over partitions 0..63
            # and row 2k+1 over 64..127
            sel_f = glob.tile([16, KT * 128], f32, name="sel_f")
            nc.gpsimd.dma_start(out=sel_f[:], in_=selm[:, :])
            sel_all = glob.tile([16, KT * 128], f32r, name="sel_all")
            nc.vector.tensor_copy(out=sel_all[:], in_=sel_f[:])
            sel = [sel_all[:, k * 128 : (k + 1) * 128] for k in range(KT)]

            # collective buffers + warmup (fires the ncfw path early)
            warm_in = dram_pool.tile([NCORES, 16], bf16, name="warm_in")
            warm_out = dram_pool.tile([NCORES, 16], bf16, name="warm_out")
            nc.gpsimd.dma_start(out=warm_in[0:1, :], in_=ones_tmp[0:1, 0:16])
            nc.gpsimd.dma_start(
                out=warm_in[1:NCORES, :],
                in_=warm_in[0:1, :].to_broadcast((NCORES - 1, 16)),
            )
            nc.gpsimd.collective_compute(
                "AllToAll",
                mybir.AluOpType.bypass,
                replica_groups=[list(range(NCORES))],
                ins=[warm_in[:].opt()],
                outs=[warm_out[:].opt()],
            )
            a2a_in = [
                dram_pool.tile([NCORES * SLICE, HB], bf16, name=f"a2a_in{b}")
                for b in range(B)
            ]
            a2a_out = [
                dram_pool.tile([NCORES * SLICE, HB], bf16, name=f"a2a_out{b}")
                for b in range(B)
            ]

            # ---------- helpers ----------
            def emit_xt_dmas(g, split=False, halves=False):
                xT_g = [
                    xT_pool.tile([128, 1024], bf16, name=f"xT_{k}", tag=f"xT{k}")
                    for k in range(KT)
                ]
                if halves:
                    # two half-column DMAs per chunk: the h=0 projection
                    # chains only wait for the first half-sized transfers
                    for h in range(2):
                        for k in range(KT):
                            eng = nc.scalar if k % 2 else nc.sync
                            c0 = g * 1024 + h * 512
                            eng.dma_start(
                                out=xT_g[k][:, h * 512 : (h + 1) * 512],
                                in_=xt[k * 128 : (k + 1) * 128, c0 : c0 + 512],
                            )
                    return xT_g
                for k in range(KT):
                    # split=True alternates trigger queues (SP/ACT HWDGE) to
                    # halve the ~600ns-per-DMA descriptor-gen serialization
                    eng = nc.scalar if (split and k % 2) else nc.sync
                    eng.dma_start(
                        out=xT_g[k][:],
                        in_=xt[k * 128 : (k + 1) * 128, g * 1024 : (g + 1) * 1024],
                    )
                return xT_g

            def emit_chain(g, h, m, xT_g):
                # one projection chain (512 rows, one of q/k/v) + v transposes
                col0 = g * 1024 + h * 512
                pp = ps_pp.tile([128, 512], f32, name="pp", tag="pp")
                for kc in range(KT):
                    nc.tensor.matmul(
                        pp[:],
                        lhsT=wqkv_sb[kc][:, m * 128 : (m + 1) * 128],
                        rhs=xT_g[kc][:, h * 512 : (h + 1) * 512],
                        start=(kc == 0),
                        stop=(kc == KT - 1),
                    )
                if m == 1:
                    nc.vector.tensor_copy(out=kT[:, col0 : col0 + 512], in_=pp[:])
                elif m == 0:
                    nc.vector.tensor_scalar_add(
                        out=qT[:, col0 : col0 + 512],
                        in0=pp[:],
                        scalar1=bias_q[:, 0:1],
                    )
                else:
                    vT_blk = vt_pool.tile(
                        [128, 512], f32, name="vT_blk", tag="vT_blk"
                    )
                    nc.vector.tensor_copy(out=vT_blk[:], in_=pp[:])
                    for j in range(4):
                        row0 = col0 + j * 128
                        b_idx = row0 // S
                        kt = (row0 % S) // 128
                        pstv = ps_pp.tile([128, 512], f32, name="pstv", tag="pp")
                        nc.tensor.transpose(
                            pstv[:, 0:128],
                            vT_blk[:, j * 128 : (j + 1) * 128],
                            ident32[:],
                        )
                        for hh in range(HPC):
                            col = ((b_idx * HPC + hh) * NKT + kt) * 65
                            nc.vector.tensor_copy(
                                out=v_nat[:, col : col + 64],
                                in_=pstv[:, hh * 64 : (hh + 1) * 64],
                            )

            def emit_attn_block(b, qb, mid=None):
                # mid: {kt: callable} emission callbacks interleaved into the
                # kt loop (just-in-time projection chains for later blocks)
                q0 = b * S + qb * 512
                po = ps_o.tile([65, 1024], f32, name="po", tag="po")
                for kt in range(NKT):
                    if mid and kt in mid:
                        mid[kt]()
                    k0 = b * S + kt * 128
                    pst = ps_st.tile([128, 1024], f32, name="pst", tag="pst")
                    for hh in range(HPC):
                        nc.tensor.matmul(
                            pst[:, hh * 512 : (hh + 1) * 512],
                            lhsT=kT[hh * 64 : (hh + 1) * 64, k0 : k0 + 128],
                            rhs=qT[hh * 64 : (hh + 1) * 64, q0 : q0 + 512],
                            start=True,
                            stop=True,
                            tile_position=(hh * 64, 0),
                        )
                    ptile = pt_pool.tile([128, 1024], bf16, name="ptile", tag="ptile")
                    nc.scalar.activation(ptile[:], pst[:], EXP, scale=SCALE)
                    for hh in range(HPC):
                        col = ((b * HPC + hh) * NKT + kt) * 65
                        nc.tensor.matmul(
                            po[:, hh * 512 : (hh + 1) * 512],
                            lhsT=v_nat[:, col : col + 65],
                            rhs=ptile[:, hh * 512 : (hh + 1) * 512],
                            start=(kt == 0),
                            stop=(kt == NKT - 1),
                        )
                # evacuate unnormalized block + denominators into the a2a
                # input, split into two 256-query half-block slices
                obk = ob_pool.tile([65, 1024], bf16, name="obk", tag="obk")
                nc.vector.tensor_copy(out=obk[:], in_=po[:])
                buf = a2a_in[b]
                # all staging on the SP queue: the POOL queue must stay clear
                # of work that queues behind a collective's completion-wait,
                # and the ACT queue must stay clear during attention
                for s in range(2):
                    r0 = (qb * 2 + s) * SLICE
                    c0 = s * HB
                    nc.sync.dma_start(
                        out=buf[r0 : r0 + 64, :], in_=obk[0:64, c0 : c0 + HB]
                    )
                    nc.sync.dma_start(
                        out=buf[r0 + 64 : r0 + 128, :],
                        in_=obk[0:64, 512 + c0 : 512 + c0 + HB],
                    )
                    nc.sync.dma_start(
                        out=buf[r0 + 128 : r0 + 129, :], in_=obk[64:65, c0 : c0 + HB]
                    )
                    nc.sync.dma_start(
                        out=buf[r0 + 129 : r0 + 130, :],
                        in_=obk[64:65, 512 + c0 : 512 + c0 + HB],
                    )
                return obk

            def emit_outproj(ph, gate=None):
                # one 256-query half: loads, reciprocal, broadcast, scale,
                # projection. Trigger queues are spread (SP/ACT/POOL) because
                # each [128,x] dma_start costs ~600ns of descriptor-gen.
                src = a2a_out[ph]
                c0 = ph * HB
                # ph 0 runs during attention: keep its triggers off the ACT
                # queue. ph 1 runs in the idle tail: spread across SP + ACT.
                tail = ph == 1
                oTs = []
                for k in range(KT):
                    o_t = glob.tile([128, HB], bf16, name=f"oTs_{ph}_{k}")
                    eng = nc.scalar if (tail and k % 2) else nc.sync
                    eng.dma_start(
                        out=o_t[:], in_=src[k * SLICE : k * SLICE + 128, :]
                    )
                    oTs.append(o_t)
                den = glob.tile([16, HB], bf16, name=f"den_{ph}")
                if gate is not None:
                    # WAW gate: forces this phase's normalize/projection work
                    # to schedule after the last attention block's evacuation
                    # (the static scheduler otherwise hoists it mid-attention
                    # and stalls the PE on the collective's latency)
                    nc.vector.tensor_copy(
                        out=den[0:1, 0:1], in_=gate[64:65, 0:1]
                    )
                for k in range(KT):
                    eng = nc.scalar if (tail and k % 2 == 0) else nc.sync
                    eng.dma_start(
                        out=den[k * HPC : (k + 1) * HPC, :],
                        in_=src[k * SLICE + 128 : k * SLICE + 130, :],
                    )
                lden = glob.tile([16, HB], f32, name=f"lden_{ph}")
                nc.scalar.activation(lden[:], den[:], LN)
                recip = glob.tile([16, HB], f32r, name=f"recip_{ph}")
                with nc.allow_low_precision(reason="softmax denom reciprocal"):
                    nc.scalar.activation(recip[:], lden[:], EXP, scale=-1.0)
                for k in range(KT):
                    pbc = ps_pp.tile([128, 512], f32, name="pbc", tag="pp")
                    nc.tensor.matmul(
                        pbc[:, 0:HB],
                        lhsT=sel[k],
                        rhs=recip[:],
                        start=True,
                        stop=True,
                    )
                    nc.vector.tensor_mul(
                        out=oTs[k][:], in0=oTs[k][:], in1=pbc[:, 0:HB]
                    )
                for m in range(8):
                    pout = ps_pp.tile([128, 512], f32, name="pout", tag="pp")
                    for k in range(KT):
                        nc.tensor.matmul(
                            pout[:, 0:HB],
                            lhsT=wo_sb[k][:, m * 128 : (m + 1) * 128],
                            rhs=oTs[k][:],
                            start=(k == 0),
                            stop=(k == KT - 1),
                        )
                    o_sb = out_pool.tile([128, 512], f32, name="o_sb", tag="o_sb")
                    nc.vector.tensor_scalar_add(
                        out=o_sb[:, 0:HB], in0=pout[:, 0:HB],
                        scalar1=bias_o[:, m : m + 1],
                    )
                    eng = nc.scalar if (tail and m % 2) else nc.sync
                    eng.dma_start(
                        out=out[m * 128 : (m + 1) * 128, c0 : c0 + HB],
                        in_=o_sb[:, 0:HB],
                    )

            # ---------- stage 1: proj b0 + attention b0 --------------------
            xg0 = emit_xt_dmas(0, halves=True)
            xg1 = emit_xt_dmas(1, split=True)
            for g in (0, 1):
                for h in range(2):
                    for m in (1, 2, 0):
                        emit_chain(g, h, m, xg0 if g == 0 else xg1)
            xg2 = emit_xt_dmas(2)
            xg3 = emit_xt_dmas(3)
            for qb in range(NQB):
                emit_attn_block(0, qb)
            # wo loads trigger during attention (POOL queue is idle here)
            wo_sb = []
            for k in range(KT):
                w_t = glob.tile([128, DIM], bf16, name=f"wo_{k}")
                nc.gpsimd.dma_start(out=w_t[:], in_=wo[k * 128 : (k + 1) * 128, :])
                wo_sb.append(w_t)
            nc.gpsimd.collective_compute(
                "AllToAll",
                mybir.AluOpType.bypass,
                replica_groups=[list(range(NCORES))],
                ins=[a2a_in[0][:].opt()],
                outs=[a2a_out[0][:].opt()],
            )

            # ---------- stage 2: proj b1 + attention b1, outproj#0 hidden --
            for g in (2, 3):
                for h in range(2):
                    for m in (1, 2, 0):
                        emit_chain(g, h, m, xg2 if g == 2 else xg3)
            emit_attn_block(1, 0)
            emit_attn_block(1, 1)
            emit_attn_block(1, 2)
            last_obk = emit_attn_block(1, 3)

            # ---------- stage 4: final exchange (hidden under outproj#0) ---
            nc.gpsimd.collective_compute(
                "AllToAll",
                mybir.AluOpType.bypass,
                replica_groups=[list(range(NCORES))],
                ins=[a2a_in[1][:].opt()],
                outs=[a2a_out[1][:].opt()],
            )
            emit_outproj(0, gate=last_obk)
            emit_outproj(1)

    nc.compile()
    return nc


def _get_graph():
    if "nc" not in _CACHED:
        _CACHED["nc"] = _build_graph()
    return _CACHED["nc"]


def _make_in_maps(x, wqkv, bqkv, wo, bo):
    bf = ml_dtypes.bfloat16
    x2 = np.asarray(x, dtype=np.float32).reshape(R, DIM)
    xt = np.ascontiguousarray(x2.T.astype(bf))  # [dim, b*s] bf16
    wqkv = np.asarray(wqkv, dtype=np.float32)
    bqkv = np.asarray(bqkv, dtype=np.float32)
    wo_f = np.asarray(wo, dtype=np.float32)
    wo16 = np.ascontiguousarray(wo_f.astype(bf))
    # fold the v-bias through the output projection (k-bias is dropped:
    # softmax-invariant)
    bv = bqkv[2 * DIM : 3 * DIM]
    bo_eff = np.asarray(bo, dtype=np.float32) + bv @ wo_f
    bo_f = np.ascontiguousarray(bo_eff.reshape(8, 128))
    # denominator-broadcast selectors: sel[k].T maps recip rows (2k, 2k+1)
    # onto output partitions (0..63, 64..127)
    selm = np.zeros((16, KT * 128), dtype=np.float32)
    for k in range(KT):
        selm[2 * k, k * 128 : k * 128 + 64] = 1.0
        selm[2 * k + 1, k * 128 + 64 : (k + 1) * 128] = 1.0
    selm = np.ascontiguousarray(selm)

    in_maps = []
    for c in range(NCORES):
        w_s = np.ascontiguousarray(
            np.concatenate(
                [
                    wqkv[:, c * FPC : (c + 1) * FPC],
                    wqkv[:, DIM + c * FPC : DIM + (c + 1) * FPC],
                    wqkv[:, 2 * DIM + c * FPC : 2 * DIM + (c + 1) * FPC],
                ],
                axis=1,
            ).astype(bf)
        )
        b_s = np.ascontiguousarray(
            bqkv[c * FPC : (c + 1) * FPC].reshape(1, FPC)
        )
        in_maps.append(
            {"xt": xt, "wqkv": w_s, "bq": b_s, "wo": wo16, "bo": bo_f, "selm": selm}
        )
    return in_maps


def _assemble_outs(outs):
    # core j's columns: [0:256] = b0 rows j*256..(j+1)*256,
    #                   [256:512] = b1 rows j*256..(j+1)*256
    full = np.empty((R, DIM), dtype=np.float32)
    for j in range(NCORES):
        full[j * HB : (j + 1) * HB, :] = outs[j][:, 0:HB].T
        full[S + j * HB : S + (j + 1) * HB, :] = outs[j][:, HB : 2 * HB].T
    return np.ascontiguousarray(full.reshape(B, S, DIM))


def kernel(x, wqkv, bqkv, wo, bo):
    from concourse.bass_utils import run_bass_kernel_spmd

    nc = _get_graph()
    in_maps = _make_in_maps(x, wqkv, bqkv, wo, bo)
    res = run_bass_kernel_spmd(nc, in_maps, core_ids=list(range(NCORES)))
    outs = [res.results[c]["out"] for c in range(NCORES)]  # each [1024, 512]
    return _assemble_outs(outs)
